# revision 1
# baseline (speedup 1.0000x reference)
"""Multi-head causal attention (b=4, n=2048, d_model=1024, 16 heads) on 8
Trainium2 NeuronCores.

Sharding: core c = (batch b = c//2, head-group hg = c%2); each core computes
one batch with 8 heads (tensor-parallel split of w_q/w_k/w_v by rows and w_o
by columns) and returns a partial [2048, 1024] output; host sums the two
head-group partials per batch.

Per-core device algorithm (all matmuls fp32r = 1 PE cycle/column):
  Phase 1: qT/kT = (X @ W.T).T via PE with host-transposed inputs; v in
           natural [seq, d] layout with an appended ones column (gives
           softmax denominators for free in the PV matmul).
  Phase 2: per q-tile t (512 q) and head-pair g: scores S^T[k,q] blocks via
           2-way row-tiled matmuls (dk=64 each), exp on ACT (scale=1/8,
           no max subtraction: |s|/8 < ~3), causal mask multiply on diagonal
           blocks, PV accumulation into [65, 512] PSUM (row 64 = rowsum).
           Stage O^T + rowsums to SBUF, reciprocal, PE-broadcast, rescale.
  Phase 3: O-projection out[seq, 1024] = O^T.T @ w_o_slice.T per q-tile.
"""

import numpy as np

B = 4
N = 2048
D_MODEL = 1024
DK = 64
NT = 4          # q tiles of 512
QT = 512        # q tile size
KB = 128        # key block size
N_CORES = 8

_CACHE = {}


def _round_f32r(x: np.ndarray) -> np.ndarray:
    """fp16 conversion for device inputs (RNE)."""
    return np.ascontiguousarray(x, dtype=np.float32).astype(np.float16)


def _split_sync_waits(nc, max_waits=1):
    """walrus on this image allows only 1 sync-wait command per instruction;
    hoist excess waits onto same-engine NoOps inserted just before."""
    import concourse.mybir as mybir

    n_split = 0
    for fn in nc.m.functions:
        for blk in fn.blocks:
            insts = list(blk.instructions)
            out = []
            for inst in insts:
                si = inst.sync_info
                if si is not None and len(si.on_wait) > max_waits:
                    waits = list(si.on_wait)
                    head, rest = waits[:-max_waits], waits[-max_waits:]
                    while head:
                        chunk, head = head[:max_waits], head[max_waits:]
                        nop = mybir.InstNoOp(
                            name=f"{inst.name}-ws{n_split}-{len(out)}",
                            engine=inst.engine,
                            opcode="NoOp",
                            sync_info=mybir.SyncInfo(on_wait=chunk, on_update=[]),
                            bass_nofuse=True,
                        )
                        out.append(nop)
                    si.on_wait = rest
                    n_split += 1
                out.append(inst)
            if len(out) != len(insts):
                blk.instructions = out
    return n_split


def build_nc():
    import concourse.bass as bass
    import concourse.mybir as mybir
    import concourse.tile as tile
    from concourse.bass import ts

    F32 = mybir.dt.float32
    F32R = mybir.dt.float16  # compute/storage dtype for all matmul operands
    AF = mybir.ActivationFunctionType

    nc = bass.Bass("TRN2", target_bir_lowering=False, debug=False)

    qT_d = nc.dram_tensor("qT", [D_MODEL, N], F32R, kind="ExternalInput")
    kT_d = nc.dram_tensor("kT", [D_MODEL, N], F32R, kind="ExternalInput")
    vT_d = nc.dram_tensor("vT", [D_MODEL, N], F32R, kind="ExternalInput")
    wqT_d = nc.dram_tensor("wqT", [D_MODEL, 512], F32R, kind="ExternalInput")
    wkT_d = nc.dram_tensor("wkT", [D_MODEL, 512], F32R, kind="ExternalInput")
    wvT_d = nc.dram_tensor("wvT", [D_MODEL, 512], F32R, kind="ExternalInput")
    woT_d = nc.dram_tensor("woT", [512, D_MODEL], F32R, kind="ExternalInput")
    masks_d = nc.dram_tensor("masks", [4, 128, 2 * QT], F32R, kind="ExternalInput")
    onescol_d = nc.dram_tensor("onescol", [128, 8], F32R, kind="ExternalInput")
    sel_d = nc.dram_tensor("sel", [8, 4, 128], F32R, kind="ExternalInput")
    out_d = nc.dram_tensor("out", [N, D_MODEL], F32, kind="ExternalOutput")

    with (
        tile.TileContext(nc) as tc,
        nc.allow_low_precision(reason="fp32r matmuls are intentional"),
    ):
        with (
            tc.tile_pool(name="persist", bufs=1) as persist,
            tc.tile_pool(name="pt_pool", bufs=1) as pt_pool,
            tc.tile_pool(name="outp", bufs=1) as outp,
        ):
            # ---- persistent SBUF tensors (whole-kernel lifetime) ----
            qT_all = persist.tile([128, 4, N], F32R)   # [part, m-block, seq]
            kT_all = persist.tile([128, 4, N], F32R)
            v_all = persist.tile([128, 16, 8, 65], F32R)  # [k-part, sb, head, d+1]
            onescol_sb = persist.tile([128, 8], F32R)
            sel_sb = persist.tile([8, 4, 128], F32R)
            nc.sync.dma_start(out=onescol_sb, in_=onescol_d[:, :])
            nc.sync.dma_start(out=sel_sb, in_=sel_d[:, :, :])

            # ================= Phase 1: projections =================
            with (
                tc.tile_pool(name="w1", bufs=1) as w1,
                tc.tile_pool(name="xs", bufs=8) as xs,
                tc.tile_pool(name="pp", bufs=1, space="PSUM") as pp,
            ):
                junk = w1.tile([128, 640], F32R)
                nc.vector.memset(junk, 0.0)
                pwarm = pp.tile([128, QT], F32, name="pwarm", tag="pj0", bufs=2)
                for _ in range(30):
                    nc.tensor.matmul(
                        pwarm, junk[:, 0:128], junk[:, 128:640],
                        start=True, stop=True,
                    )
                wq_sb = w1.tile([128, 8, 512], F32R)
                wk_sb = w1.tile([128, 8, 512], F32R)
                wv_sb = w1.tile([128, 8, 512], F32R)
                for kc in range(8):
                    nc.sync.dma_start(out=wq_sb[:, kc, :], in_=wqT_d[ts(kc, 128), :])

                # q/k projections: qT_all[:, m, tsl] = (W X^T) block
                for src_d, w_sb, w_src, dst in (
                    (qT_d, wq_sb, None, qT_all),
                    (kT_d, wk_sb, wkT_d, kT_all),
                ):
                    if w_src is not None:
                        for kc in range(8):
                            nc.sync.dma_start(
                                out=w_sb[:, kc, :], in_=w_src[ts(kc, 128), :]
                            )
                    for t in range(NT):
                        pj = [
                            pp.tile(
                                [128, QT], F32, name=f"pj{m}", tag=f"pj{m}", bufs=2
                            )
                            for m in range(4)
                        ]
                        for kc in range(8):
                            x_t = xs.tile([128, QT], F32R, name="x_t", tag="x_t")
                            nc.sync.dma_start(
                                out=x_t, in_=src_d[ts(kc, 128), ts(t, QT)]
                            )
                            for m in range(4):
                                nc.tensor.matmul(
                                    pj[m],
                                    w_sb[:, kc, ts(m, 128)],
                                    x_t[:, :],
                                    start=(kc == 0),
                                    stop=(kc == 7),
                                )
                        for m in range(4):
                            nc.vector.tensor_copy(dst[:, m, ts(t, QT)], pj[m])

                # v projection: natural [seq, d] layout + ones column
                for kc in range(8):
                    nc.sync.dma_start(out=wv_sb[:, kc, :], in_=wvT_d[ts(kc, 128), :])
                for t in range(NT):
                    pj = [
                        pp.tile([128, QT], F32, name=f"pj{m}", tag=f"pj{m}", bufs=2)
                        for m in range(4)
                    ]
                    for kc in range(8):
                        x_t = xs.tile([128, QT], F32R, name="x_t", tag="x_t")
                        nc.sync.dma_start(out=x_t, in_=vT_d[ts(kc, 128), ts(t, QT)])
                        for m in range(4):
                            nc.tensor.matmul(
                                pj[m],
                                x_t[:, ts(m, 128)],
                                wv_sb[:, kc, :],
                                start=(kc == 0),
                                stop=(kc == 7),
                            )
                    for m in range(4):
                        sb = t * 4 + m
                        nc.vector.tensor_copy(
                            v_all[:, sb, :, 0:64],
                            pj[m][:, :].rearrange("p (h d) -> p h d", h=8),
                        )
                        nc.vector.tensor_copy(v_all[:, sb, :, 64], onescol_sb)

            # ================= Phase 2+3: attention + O-projection =========
            with (
                tc.tile_pool(name="persist2", bufs=1) as persist2,
                tc.tile_pool(name="ps2", bufs=1, space="PSUM") as ps2,
            ):
                ot_sb = [
                    persist2.tile([128, 4, QT], F32R, name=f"ot_sb{t}", tag=f"ot{t}")
                    for t in range(NT)
                ]
                rs_sb = [
                    persist2.tile([8, QT], F32R, name=f"rs_sb{t}", tag=f"rs{t}")
                    for t in range(NT)
                ]
                recip_sb = [
                    persist2.tile([8, QT], F32R, name=f"recip{t}", tag=f"rc{t}")
                    for t in range(NT)
                ]
                masks_sb = persist2.tile([128, 4, 2 * QT], F32R)
                wo_sb = persist2.tile([128, 4, D_MODEL], F32R)

                for r in range(4):
                    nc.sync.dma_start(out=masks_sb[:, r, :], in_=masks_d[r, :, :])
                for g in range(4):
                    nc.sync.dma_start(out=wo_sb[:, g, :], in_=woT_d[ts(g, 128), :])

                pending = []
                for t in range(NT):
                    nkb = 4 * t + 4  # causal: key blocks 0 .. 4t+3
                    for g in range(4):
                        ota = ps2.tile([65, QT], F32, name="ota", tag="ota", bufs=1)
                        otb = ps2.tile([65, QT], F32, name="otb", tag="otb", bufs=1)
                        for j in range(nkb):
                            sp = ps2.tile(
                                [128, 2 * QT], F32, name="sp", tag="sp", bufs=2
                            )
                            nc.tensor.matmul(
                                sp[:, 0:QT],
                                kT_all[0:64, g, ts(j, 128)],
                                qT_all[0:64, g, ts(t, QT)],
                                start=True,
                                stop=True,
                                tile_position=(0, 0),
                            )
                            nc.tensor.matmul(
                                sp[:, QT : 2 * QT],
                                kT_all[64:128, g, ts(j, 128)],
                                qT_all[64:128, g, ts(t, QT)],
                                start=True,
                                stop=True,
                                tile_position=(64, 0),
                            )
                            pt2 = pt_pool.tile(
                                [128, 2 * QT], F32R, name="pt2", tag="pt2", bufs=6
                            )
                            if g == 2 and j == 2 and pending:
                                pending.pop(0)()
                            nc.scalar.activation(pt2, sp, AF.Exp, scale=0.125)
                            r = j - 4 * t
                            if r >= 0:
                                nc.vector.tensor_mul(
                                    pt2, pt2, masks_sb[:, r, :]
                                )
                            z = 128 * r if r > 0 else 0
                            nc.tensor.matmul(
                                ota[:, z:QT],
                                v_all[:, j, 2 * g, :],
                                pt2[:, z:QT],
                                start=(j == 0),
                                stop=(j == nkb - 1),
                            )
                            nc.tensor.matmul(
                                otb[:, z:QT],
                                v_all[:, j, 2 * g + 1, :],
                                pt2[:, QT + z : 2 * QT],
                                start=(j == 0),
                                stop=(j == nkb - 1),
                            )
                        # stage O^T and rowsums to SBUF
                        nc.vector.tensor_copy(ot_sb[t][0:64, g, :], ota[0:64, :])
                        nc.vector.tensor_copy(ot_sb[t][64:128, g, :], otb[0:64, :])
                        tmp_rs = pt_pool.tile(
                            [1, 2, QT], F32R, name="tmp_rs", tag="tmp_rs", bufs=2
                        )
                        nc.vector.tensor_copy(tmp_rs[0:1, 0, :], ota[64:65, :])
                        nc.vector.tensor_copy(tmp_rs[0:1, 1, :], otb[64:65, :])
                        nc.sync.dma_start(
                            out=rs_sb[t][2 * g : 2 * g + 2, :], in_=tmp_rs[0:1, :, :]
                        )

                    # normalize this q-tile: recip of gathered rowsums,
                    # PE-broadcast per head pair via K=8 selector matmul
                    def normalize_and_oproj(t=t):
                        nc.vector.reciprocal(recip_sb[t], rs_sb[t])
                        for g in range(4):
                            bc = ps2.tile(
                                [128, QT], F32, name="bc", tag="aux", bufs=2
                            )
                            nc.tensor.matmul(
                                bc,
                                sel_sb[:, g, :],
                                recip_sb[t][:, :],
                                start=True,
                                stop=True,
                            )
                            nc.vector.tensor_mul(
                                ot_sb[t][:, g, :], ot_sb[t][:, g, :], bc
                            )
                        for mm in range(4):
                            m = 4 * t + mm
                            for n2 in range(2):
                                po = ps2.tile(
                                    [128, 512], F32, name="po", tag="aux", bufs=2
                                )
                                for g in range(4):
                                    nc.tensor.matmul(
                                        po,
                                        ot_sb[t][:, g, ts(mm, 128)],
                                        wo_sb[:, g, ts(n2, 512)],
                                        start=(g == 0),
                                        stop=(g == 3),
                                    )
                                ob = outp.tile(
                                    [128, 512], F32, name="ob", tag="ob", bufs=3
                                )
                                nc.vector.tensor_copy(ob, po)
                                nc.sync.dma_start(
                                    out=out_d[ts(m, 128), ts(n2, 512)], in_=ob
                                )

                    pending.append(normalize_and_oproj)
                for fn in pending:
                    fn()

    _split_sync_waits(nc)
    return nc


def _prep_inputs(Q, K, V, w_q, w_k, w_v, w_o):
    """Build the 8 per-core input maps (host-side shard + transpose + f32r)."""
    Q = np.asarray(Q, dtype=np.float32)
    K = np.asarray(K, dtype=np.float32)
    V = np.asarray(V, dtype=np.float32)
    w_q = np.asarray(w_q, dtype=np.float32)
    w_k = np.asarray(w_k, dtype=np.float32)
    w_v = np.asarray(w_v, dtype=np.float32)
    w_o = np.asarray(w_o, dtype=np.float32)

    masks = np.zeros((4, 128, 2 * QT), dtype=np.float16)
    k_idx = np.arange(128)[:, None]
    q_idx = np.arange(QT)[None, :]
    for r in range(4):
        m = (k_idx <= q_idx - 128 * r).astype(np.float16)
        masks[r, :, 0:QT] = m
        masks[r, :, QT:] = m
    onescol = np.ones((128, 8), dtype=np.float16)
    sel = np.zeros((8, 4, 128), dtype=np.float16)
    for g in range(4):
        sel[2 * g, g, 0:64] = 1.0
        sel[2 * g + 1, g, 64:128] = 1.0

    qT = [_round_f32r(Q[b].T) for b in range(B)]
    kT = [_round_f32r(K[b].T) for b in range(B)]
    vT = [_round_f32r(V[b].T) for b in range(B)]
    wqT = [_round_f32r(w_q[hg * 512 : hg * 512 + 512, :].T) for hg in range(2)]
    wkT = [_round_f32r(w_k[hg * 512 : hg * 512 + 512, :].T) for hg in range(2)]
    wvT = [_round_f32r(w_v[hg * 512 : hg * 512 + 512, :].T) for hg in range(2)]
    woT = [_round_f32r(w_o[:, hg * 512 : hg * 512 + 512].T) for hg in range(2)]

    in_maps = []
    for c in range(N_CORES):
        b, hg = c // 2, c % 2
        in_maps.append(
            {
                "qT": qT[b],
                "kT": kT[b],
                "vT": vT[b],
                "wqT": wqT[hg],
                "wkT": wkT[hg],
                "wvT": wvT[hg],
                "woT": woT[hg],
                "masks": masks,
                "onescol": onescol,
                "sel": sel,
            }
        )
    return in_maps


def kernel(Q, K, V, w_q, w_k, w_v, w_o, _trace=False):
    from concourse.bass_utils import run_bass_kernel_spmd

    if "nc" not in _CACHE:
        _CACHE["nc"] = build_nc()
    nc = _CACHE["nc"]

    in_maps = _prep_inputs(Q, K, V, w_q, w_k, w_v, w_o)
    res = run_bass_kernel_spmd(
        nc, in_maps, core_ids=list(range(N_CORES)), trace=_trace
    )
    outs = [r["out"] for r in res.results]
    full = np.empty((B, N, D_MODEL), dtype=np.float32)
    for b in range(B):
        full[b] = outs[2 * b] + outs[2 * b + 1]
    if _trace:
        _CACHE["last_result"] = res
    return full



# revision 2
# speedup vs baseline: 1.0887x; 1.0887x over previous
"""Multi-head causal attention (b=4, n=2048, d_model=1024, 16 heads) on 8
Trainium2 NeuronCores.

Sharding: core c = (batch b = c//2, head-group hg = c%2); each core computes
one batch with 8 heads (tensor-parallel split of w_q/w_k/w_v by rows and w_o
by columns) and returns a partial [2048, 1024] output; host sums the two
head-group partials per batch.

v2 vs baseline:
- Scores run in fp8e4 DoubleRow mode (0.5 PE cycles/row): q/k projections
  stay fp16 for accuracy, but their outputs are cast straight to fp8 and
  regrouped (flat sbuf->sbuf DMA, [128,512] -> [64,2,512]) into the
  DoubleRow pairing d = 2p + i.
- Scores/exp/PV are trimmed to the causal window on diagonal blocks
  (baseline only trimmed PV).
- The causal mask multiply runs on the idle GpSimd engine against a single
  [128,2,128] triangular window instead of DVE x [128,1024].
- Projection and O-projection PE work is woven into the attention ladder
  as filler units so the PE never idles waiting on exp; ACT (exp) and PE
  stay concurrently busy instead of phase-serialized.
"""

from collections import deque

import numpy as np

B = 4
N = 2048
D_MODEL = 1024
DK = 64
NT = 4          # q tiles of 512
QT = 512        # q tile size
N_CORES = 8

_CACHE = {}


def _split_sync_waits(nc, max_waits=1):
    """walrus on this image allows only 1 sync-wait command per instruction;
    hoist excess waits onto same-engine NoOps inserted just before."""
    import concourse.mybir as mybir

    n_split = 0
    for fn in nc.m.functions:
        for blk in fn.blocks:
            insts = list(blk.instructions)
            out = []
            for inst in insts:
                si = inst.sync_info
                if si is not None and len(si.on_wait) > max_waits:
                    waits = list(si.on_wait)
                    head, rest = waits[:-max_waits], waits[-max_waits:]
                    while head:
                        chunk, head = head[:max_waits], head[max_waits:]
                        nop = mybir.InstNoOp(
                            name=f"{inst.name}-ws{n_split}-{len(out)}",
                            engine=inst.engine,
                            opcode="NoOp",
                            sync_info=mybir.SyncInfo(on_wait=chunk, on_update=[]),
                            bass_nofuse=True,
                        )
                        out.append(nop)
                    si.on_wait = rest
                    n_split += 1
                out.append(inst)
            if len(out) != len(insts):
                blk.instructions = out
    return n_split


def build_nc():
    import concourse.bass as bass
    import concourse.mybir as mybir
    import concourse.tile as tile
    from concourse.bass import ts

    F32 = mybir.dt.float32
    F16 = mybir.dt.float16
    F8 = mybir.dt.float8e4
    AF = mybir.ActivationFunctionType
    DR = mybir.MatmulPerfMode.DoubleRow

    nc = bass.Bass("TRN2", target_bir_lowering=False, debug=False)

    qT_d = nc.dram_tensor("qT", [D_MODEL, N], F16, kind="ExternalInput")
    kT_d = nc.dram_tensor("kT", [D_MODEL, N], F16, kind="ExternalInput")
    vT_d = nc.dram_tensor("vT", [D_MODEL, N], F16, kind="ExternalInput")
    wqT_d = nc.dram_tensor("wqT", [D_MODEL, 512], F16, kind="ExternalInput")
    wkT_d = nc.dram_tensor("wkT", [D_MODEL, 512], F16, kind="ExternalInput")
    wvT_d = nc.dram_tensor("wvT", [D_MODEL, 512], F16, kind="ExternalInput")
    woT_d = nc.dram_tensor("woT", [512, D_MODEL], F16, kind="ExternalInput")
    maskw_d = nc.dram_tensor("maskw", [128, 2, 128], F16, kind="ExternalInput")
    onescol_d = nc.dram_tensor("onescol", [128, 8], F16, kind="ExternalInput")
    sel_d = nc.dram_tensor("sel", [8, 4, 128], F16, kind="ExternalInput")
    out_d = nc.dram_tensor("out", [N, D_MODEL], F32, kind="ExternalOutput")

    with (
        tile.TileContext(nc) as tc,
        nc.allow_low_precision(reason="fp8/fp16 matmuls are intentional"),
    ):
        with (
            tc.tile_pool(name="persist", bufs=1) as persist,
            tc.tile_pool(name="pt_pool", bufs=1) as pt_pool,
            tc.tile_pool(name="xs", bufs=2) as xs,
            tc.tile_pool(name="outp", bufs=1) as outp,
            tc.tile_pool(name="ps", bufs=1, space="PSUM") as ps,
        ):
            # ---- persistent SBUF tensors ----
            # [32*b2+p, g, i, n]: head 2g+b2 at partitions 32*b2..32*b2+31,
            # plane g; DoubleRow contraction pairing d = 2p + i
            qdr = persist.tile([64, 4, 2, N], F8)
            kdr = persist.tile([64, 4, 2, N], F8)
            qT8 = persist.tile([128, 4, N], F8)      # natural layout staging
            kT8 = persist.tile([128, 4, N], F8)
            v_all = persist.tile([128, 16, 8, 65], F16)  # [key, sb, head, d+1]
            maskw_sb = persist.tile([128, 2, 128], F16)
            onescol_sb = persist.tile([128, 8], F16)
            sel_sb = persist.tile([8, 4, 128], F16)
            wq_sb = persist.tile([128, 8, 512], F16)
            wk_sb = persist.tile([128, 8, 512], F16)
            wv_sb = persist.tile([128, 8, 512], F16)
            wo_sb = persist.tile([128, 4, D_MODEL], F16)
            ot_sb = [
                persist.tile([128, 4, QT], F16, name=f"ot_sb{t}", tag=f"ot{t}")
                for t in range(NT)
            ]
            rs_sb = [
                persist.tile([8, QT], F32, name=f"rs_sb{t}", tag=f"rs{t}")
                for t in range(NT)
            ]
            recip_sb = [
                persist.tile([8, QT], F16, name=f"recip{t}", tag=f"rc{t}")
                for t in range(NT)
            ]

            # DMA order matters at startup: the v-projection path (wv + vT
            # tile 0) is needed first; wo only at the first O-projection.
            for kc in range(8):
                nc.sync.dma_start(out=wv_sb[:, kc, :], in_=wvT_d[ts(kc, 128), :])
            nc.sync.dma_start(out=maskw_sb, in_=maskw_d[:, :, :])
            nc.sync.dma_start(out=onescol_sb, in_=onescol_d[:, :])
            nc.sync.dma_start(out=sel_sb, in_=sel_d[:, :, :])

            # ---- PE p-state warmup while initial DMAs land ----
            junk = persist.tile([128, 640], F16)
            nc.vector.memset(junk, 0.0)
            pwarm = ps.tile([128, QT], F32, name="pwarm", tag="pj", bufs=2)
            for _ in range(14):
                nc.tensor.matmul(
                    pwarm, junk[:, 0:128], junk[:, 128:640], start=True, stop=True
                )

            # ---------- projection emitters (per q/k/v tile m-block) -------
            def load_x_tiles(t):
                """DMA the x operand tiles for projection tile t (v first —
                its units run first)."""
                tsl = ts(t, QT)
                tiles = {}
                for key, src_d in (("v", vT_d), ("k", kT_d), ("q", qT_d)):
                    x = xs.tile(
                        [128, 8, QT], F16, name=f"x{key}", tag=f"x{key}", bufs=2
                    )
                    for kc in range(8):
                        nc.sync.dma_start(out=x[:, kc, :], in_=src_d[ts(kc, 128), tsl])
                    tiles[key] = x
                return tiles

            def proj_qk_unit(which, x, t, m):
                """One m-block of the q or k projection: fp16 matmuls,
                cast to fp8, regroup-DMA into DoubleRow layout."""
                tsl = ts(t, QT)
                w_sb = wq_sb if which == "q" else wk_sb
                nat = qT8 if which == "q" else kT8
                dr = qdr if which == "q" else kdr
                pj = ps.tile([128, QT], F32, name="pj", tag="pj", bufs=2)
                for kc in range(8):
                    nc.tensor.matmul(
                        pj,
                        w_sb[:, kc, ts(m, 128)],
                        x[:, kc, :],
                        start=(kc == 0),
                        stop=(kc == 7),
                    )
                nc.vector.tensor_copy(nat[:, m, tsl], pj)
                nc.sync.dma_start(out=dr[0:64, m, :, tsl], in_=nat[:, m, tsl])

            def proj_v_unit(x, t, m):
                """One m-block of the v projection (natural [seq, d] layout
                plus the ones column feeding softmax denominators)."""
                pj = ps.tile([128, QT], F32, name="pj", tag="pj", bufs=2)
                for kc in range(8):
                    nc.tensor.matmul(
                        pj,
                        x[:, kc, ts(m, 128)],
                        wv_sb[:, kc, :],
                        start=(kc == 0),
                        stop=(kc == 7),
                    )
                sb = t * 4 + m
                nc.vector.tensor_copy(
                    v_all[:, sb, :, 0:64],
                    pj[:, :].rearrange("p (h d) -> p h d", h=8),
                )
                nc.vector.tensor_copy(v_all[:, sb, :, 64], onescol_sb)

            def proj_tile_units(t, x=None):
                """Filler units projecting tile t (v first: attention tile t
                consumes v_all[4t..4t+3] earliest via PV j=4t)."""
                if x is None:
                    x = load_x_tiles(t)
                units = []
                for m in range(4):
                    units.append(lambda m=m, x=x["v"]: proj_v_unit(x, t, m))
                for m in range(4):
                    units.append(lambda m=m, x=x["k"]: proj_qk_unit("k", x, t, m))
                for m in range(4):
                    units.append(lambda m=m, x=x["q"]: proj_qk_unit("q", x, t, m))
                return units

            # ---------- normalize + O-projection emitters ------------------
            def norm_unit(t):
                nc.vector.reciprocal(recip_sb[t], rs_sb[t])
                for g in range(4):
                    bc = ps.tile([128, QT], F32, name="bc", tag="pj", bufs=2)
                    nc.tensor.matmul(
                        bc, sel_sb[:, g, :], recip_sb[t][:, :],
                        start=True, stop=True,
                    )
                    nc.vector.tensor_mul(ot_sb[t][:, g, :], ot_sb[t][:, g, :], bc)

            def oproj_unit(t, mm, n2):
                po = ps.tile([128, 512], F32, name="po", tag="pj", bufs=2)
                for g in range(4):
                    nc.tensor.matmul(
                        po,
                        ot_sb[t][:, g, ts(mm, 128)],
                        wo_sb[:, g, ts(n2, 512)],
                        start=(g == 0),
                        stop=(g == 3),
                    )
                ob = outp.tile([128, 512], F32, name="ob", tag="ob", bufs=3)
                nc.vector.tensor_copy(ob, po)
                nc.sync.dma_start(
                    out=out_d[ts(4 * t + mm, 128), ts(n2, 512)], in_=ob
                )

            def oproj_tile_units(t):
                units = [lambda: norm_unit(t)]
                for mm in range(4):
                    for n2 in range(2):
                        units.append(
                            lambda mm=mm, n2=n2: oproj_unit(t, mm, n2)
                        )
                return units

            # ================= main schedule =================
            # proj(t0) up front; proj(t+1) and oproj(t-1) woven into the
            # attention ladder as PE filler between blocks.
            filler = deque()
            # tile-0 x DMAs go out before the q/k/o weight DMAs so the
            # v-projection (first PE work after warmup) is fed earliest
            x0 = load_x_tiles(0)
            for kc in range(8):
                nc.sync.dma_start(out=wq_sb[:, kc, :], in_=wqT_d[ts(kc, 128), :])
                nc.sync.dma_start(out=wk_sb[:, kc, :], in_=wkT_d[ts(kc, 128), :])
            for g in range(4):
                nc.sync.dma_start(out=wo_sb[:, g, :], in_=woT_d[ts(g, 128), :])
            for u in proj_tile_units(0, x0):
                u()

            for t in range(NT):
                nkb = 4 * t + 4  # causal: key blocks 0 .. 4t+3
                if t + 1 < NT:
                    filler.extend(proj_tile_units(t + 1))
                if t > 0:
                    filler.extend(oproj_tile_units(t - 1))
                blocks_left = 4 * nkb
                stride = max(1, (4 * nkb) // max(1, len(filler)))
                since = 0
                for g in range(4):
                    ota = ps.tile([65, QT], F32, name="ota", tag="ota", bufs=1)
                    otb = ps.tile([65, QT], F32, name="otb", tag="otb", bufs=1)
                    for j in range(nkb):
                        r = j - 4 * t
                        z = 128 * r if r > 0 else 0
                        sp = ps.tile(
                            [128, 2, QT], F32, name="sp", tag="sp", bufs=2
                        )
                        for b2 in range(2):
                            pb = 32 * b2
                            nc.tensor.matmul(
                                sp[:, b2, z:QT],
                                kdr[pb : pb + 32, g, :, ts(j, 128)],
                                qdr[pb : pb + 32, g, :, t * QT + z : (t + 1) * QT],
                                start=True,
                                stop=True,
                                perf_mode=DR,
                            )
                        pt2 = pt_pool.tile(
                            [128, 2, QT], F16, name="pt2", tag="pt2", bufs=6
                        )
                        nc.scalar.activation(
                            pt2[:, :, z:QT], sp[:, :, z:QT], AF.Exp, scale=0.125
                        )
                        if r >= 0:
                            nc.gpsimd.tensor_mul(
                                pt2[:, :, z : z + 128],
                                pt2[:, :, z : z + 128],
                                maskw_sb,
                            )
                        nc.tensor.matmul(
                            ota[:, z:QT],
                            v_all[:, j, 2 * g, :],
                            pt2[:, 0, z:QT],
                            start=(j == 0),
                            stop=(j == nkb - 1),
                        )
                        nc.tensor.matmul(
                            otb[:, z:QT],
                            v_all[:, j, 2 * g + 1, :],
                            pt2[:, 1, z:QT],
                            start=(j == 0),
                            stop=(j == nkb - 1),
                        )
                        # weave pending proj/oproj PE work between blocks,
                        # spread evenly and guaranteed drained by tile end
                        since += 1
                        if filler and since >= stride:
                            filler.popleft()()
                            since = 0
                        while filler and len(filler) >= blocks_left:
                            filler.popleft()()
                        blocks_left -= 1
                    # stage O^T and rowsums to SBUF
                    nc.vector.tensor_copy(ot_sb[t][0:64, g, :], ota[0:64, :])
                    nc.vector.tensor_copy(ot_sb[t][64:128, g, :], otb[0:64, :])
                    tmp_rs = pt_pool.tile(
                        [1, 2, QT], F32, name="tmp_rs", tag="tmp_rs", bufs=2
                    )
                    nc.vector.tensor_copy(tmp_rs[0:1, 0, :], ota[64:65, :])
                    nc.vector.tensor_copy(tmp_rs[0:1, 1, :], otb[64:65, :])
                    nc.sync.dma_start(
                        out=rs_sb[t][2 * g : 2 * g + 2, :], in_=tmp_rs[0:1, :, :]
                    )
            for u in filler:
                u()
            for u in oproj_tile_units(NT - 1):
                u()

    _split_sync_waits(nc)
    return nc


def _prep_inputs(Q, K, V, w_q, w_k, w_v, w_o):
    """Build the 8 per-core input maps (host-side shard + transpose + cast)."""
    Q = np.asarray(Q, dtype=np.float32)
    K = np.asarray(K, dtype=np.float32)
    V = np.asarray(V, dtype=np.float32)
    w_q = np.asarray(w_q, dtype=np.float32)
    w_k = np.asarray(w_k, dtype=np.float32)
    w_v = np.asarray(w_v, dtype=np.float32)
    w_o = np.asarray(w_o, dtype=np.float32)

    k_idx = np.arange(128)[:, None]
    w_idx = np.arange(128)[None, :]
    maskw = np.zeros((128, 2, 128), dtype=np.float16)
    maskw[:, 0, :] = (k_idx <= w_idx).astype(np.float16)
    maskw[:, 1, :] = maskw[:, 0, :]
    onescol = np.ones((128, 8), dtype=np.float16)
    sel = np.zeros((8, 4, 128), dtype=np.float16)
    for g in range(4):
        sel[2 * g, g, 0:64] = 1.0
        sel[2 * g + 1, g, 64:128] = 1.0

    def f16T(a):
        return np.ascontiguousarray(a.T).astype(np.float16)

    qT = [f16T(Q[b]) for b in range(B)]
    kT = [f16T(K[b]) for b in range(B)]
    vT = [f16T(V[b]) for b in range(B)]
    wqT = [f16T(w_q[hg * 512 : hg * 512 + 512, :]) for hg in range(2)]
    wkT = [f16T(w_k[hg * 512 : hg * 512 + 512, :]) for hg in range(2)]
    wvT = [f16T(w_v[hg * 512 : hg * 512 + 512, :]) for hg in range(2)]
    woT = [f16T(w_o[:, hg * 512 : hg * 512 + 512]) for hg in range(2)]

    in_maps = []
    for c in range(N_CORES):
        b, hg = c // 2, c % 2
        in_maps.append(
            {
                "qT": qT[b],
                "kT": kT[b],
                "vT": vT[b],
                "wqT": wqT[hg],
                "wkT": wkT[hg],
                "wvT": wvT[hg],
                "woT": woT[hg],
                "maskw": maskw,
                "onescol": onescol,
                "sel": sel,
            }
        )
    return in_maps


def kernel(Q, K, V, w_q, w_k, w_v, w_o, _trace=False):
    from concourse.bass_utils import run_bass_kernel_spmd

    if "nc" not in _CACHE:
        _CACHE["nc"] = build_nc()
    nc = _CACHE["nc"]

    in_maps = _prep_inputs(Q, K, V, w_q, w_k, w_v, w_o)
    res = run_bass_kernel_spmd(
        nc, in_maps, core_ids=list(range(N_CORES)), trace=_trace
    )
    outs = [r["out"] for r in res.results]
    full = np.empty((B, N, D_MODEL), dtype=np.float32)
    for b in range(B):
        full[b] = outs[2 * b] + outs[2 * b + 1]
    if _trace:
        _CACHE["last_result"] = res
    return full


# revision 3
# speedup vs baseline: 1.0960x; 1.0067x over previous
"""Multi-head causal attention (b=4, n=2048, d_model=1024, 16 heads) on 8
Trainium2 NeuronCores.

Sharding: core c = (batch b = c//2, head-group hg = c%2); each core computes
one batch with 8 heads (tensor-parallel split of w_q/w_k/w_v by rows and w_o
by columns) and returns a partial [2048, 1024] output; host sums the two
head-group partials per batch.

v2 vs baseline:
- Scores run in fp8e4 DoubleRow mode (0.5 PE cycles/row): q/k projections
  stay fp16 for accuracy, but their outputs are cast straight to fp8 and
  regrouped (flat sbuf->sbuf DMA, [128,512] -> [64,2,512]) into the
  DoubleRow pairing d = 2p + i.
- Scores/exp/PV are trimmed to the causal window on diagonal blocks
  (baseline only trimmed PV).
- The causal mask multiply runs on the idle GpSimd engine against a single
  [128,2,128] triangular window instead of DVE x [128,1024].
- Projection and O-projection PE work is woven into the attention ladder
  as filler units so the PE never idles waiting on exp; ACT (exp) and PE
  stay concurrently busy instead of phase-serialized.
"""

from collections import deque

import numpy as np

B = 4
N = 2048
D_MODEL = 1024
DK = 64
NT = 4          # q tiles of 512
QT = 512        # q tile size
N_CORES = 8

_CACHE = {}


def _split_sync_waits(nc, max_waits=1):
    """walrus on this image allows only 1 sync-wait command per instruction;
    hoist excess waits onto same-engine NoOps inserted just before."""
    import concourse.mybir as mybir

    n_split = 0
    for fn in nc.m.functions:
        for blk in fn.blocks:
            insts = list(blk.instructions)
            out = []
            for inst in insts:
                si = inst.sync_info
                if si is not None and len(si.on_wait) > max_waits:
                    waits = list(si.on_wait)
                    head, rest = waits[:-max_waits], waits[-max_waits:]
                    while head:
                        chunk, head = head[:max_waits], head[max_waits:]
                        nop = mybir.InstNoOp(
                            name=f"{inst.name}-ws{n_split}-{len(out)}",
                            engine=inst.engine,
                            opcode="NoOp",
                            sync_info=mybir.SyncInfo(on_wait=chunk, on_update=[]),
                            bass_nofuse=True,
                        )
                        out.append(nop)
                    si.on_wait = rest
                    n_split += 1
                out.append(inst)
            if len(out) != len(insts):
                blk.instructions = out
    return n_split


def build_nc():
    import concourse.bass as bass
    import concourse.mybir as mybir
    import concourse.tile as tile
    from concourse.bass import ts

    F32 = mybir.dt.float32
    F16 = mybir.dt.float16
    F8 = mybir.dt.float8e4
    AF = mybir.ActivationFunctionType
    DR = mybir.MatmulPerfMode.DoubleRow

    nc = bass.Bass("TRN2", target_bir_lowering=False, debug=False)

    xq8_d = nc.dram_tensor("xq8", [8, 128, 2, N], F8, kind="ExternalInput")
    xk8_d = nc.dram_tensor("xk8", [8, 128, 2, N], F8, kind="ExternalInput")
    vT_d = nc.dram_tensor("vT", [D_MODEL, N], F16, kind="ExternalInput")
    wq8_d = nc.dram_tensor("wq8", [8, 128, 2, 512], F8, kind="ExternalInput")
    wk8_d = nc.dram_tensor("wk8", [8, 128, 2, 512], F8, kind="ExternalInput")
    wvT_d = nc.dram_tensor("wvT", [D_MODEL, 512], F16, kind="ExternalInput")
    woT_d = nc.dram_tensor("woT", [512, D_MODEL], F16, kind="ExternalInput")
    maskw_d = nc.dram_tensor("maskw", [128, 2, 128], F16, kind="ExternalInput")
    onescol_d = nc.dram_tensor("onescol", [128, 8], F16, kind="ExternalInput")
    sel_d = nc.dram_tensor("sel", [8, 4, 128], F16, kind="ExternalInput")
    sel6_d = nc.dram_tensor("sel6", [6, 3, 128], F16, kind="ExternalInput")
    sel2_d = nc.dram_tensor("sel2", [2, 128], F16, kind="ExternalInput")
    out_d = nc.dram_tensor("out", [N, D_MODEL], F32, kind="ExternalOutput")

    with (
        tile.TileContext(nc) as tc,
        nc.allow_low_precision(reason="fp8/fp16 matmuls are intentional"),
    ):
        with (
            tc.tile_pool(name="persist", bufs=1) as persist,
            tc.tile_pool(name="pt_pool", bufs=1) as pt_pool,
            tc.tile_pool(name="xs", bufs=2) as xs,
            tc.tile_pool(name="outp", bufs=1) as outp,
            tc.tile_pool(name="ps", bufs=1, space="PSUM") as ps,
        ):
            # ---- persistent SBUF tensors ----
            # [32*b2+p, g, i, n]: head 2g+b2 at partitions 32*b2..32*b2+31,
            # plane g; DoubleRow contraction pairing d = 2p + i
            qdr = persist.tile([64, 4, 2, N], F8)
            kdr = persist.tile([64, 4, 2, N], F8)
            qT8 = persist.tile([128, 4, N], F8)      # natural layout staging
            kT8 = persist.tile([128, 4, N], F8)
            v_all = persist.tile([128, 16, 8, 65], F16)  # [key, sb, head, d+1]
            maskw_sb = persist.tile([128, 2, 128], F16)
            onescol_sb = persist.tile([128, 8], F16)
            sel_sb = persist.tile([8, 4, 128], F16)
            sel6_sb = persist.tile([6, 3, 128], F16)
            sel2_sb = persist.tile([2, 128], F16)
            rs3b = persist.tile([2, QT], F32)
            recip3b = persist.tile([2, QT], F16)
            # q/k projection weights in fp8 with hi/lo slots (w8, w8/16):
            # paired against moving slots (x8, 16*(x - x8)) the DoubleRow
            # matmul computes w8 @ x at ~fp12 x-precision, half the PE rows
            wq8_sb = persist.tile([128, 8, 2, 512], F8)
            wk8_sb = persist.tile([128, 8, 2, 512], F8)
            wv_sb = persist.tile([128, 8, 512], F16)
            wo_sb = persist.tile([128, 4, D_MODEL], F16)
            ot_sb = [
                persist.tile([128, 4, QT], F16, name=f"ot_sb{t}", tag=f"ot{t}")
                for t in range(NT)
            ]
            rs_sb = [
                persist.tile([8, QT], F32, name=f"rs_sb{t}", tag=f"rs{t}")
                for t in range(NT)
            ]
            recip_sb = [
                persist.tile([8, QT], F16, name=f"recip{t}", tag=f"rc{t}")
                for t in range(NT)
            ]

            # DMA order matters at startup: the v-projection path (wv + vT
            # tile 0) is needed first; wo only at the first O-projection.
            for kc in range(8):
                nc.sync.dma_start(out=wv_sb[:, kc, :], in_=wvT_d[ts(kc, 128), :])
            nc.sync.dma_start(out=maskw_sb, in_=maskw_d[:, :, :])
            nc.sync.dma_start(out=onescol_sb, in_=onescol_d[:, :])
            nc.sync.dma_start(out=sel_sb, in_=sel_d[:, :, :])
            nc.sync.dma_start(out=sel6_sb, in_=sel6_d[:, :, :])
            nc.sync.dma_start(out=sel2_sb, in_=sel2_d[:, :])

            # ---- PE p-state warmup while initial DMAs land ----
            junk = persist.tile([128, 640], F16)
            nc.vector.memset(junk, 0.0)
            pwarm = ps.tile([128, QT], F32, name="pwarm", tag="pj", bufs=2)
            for _ in range(14):
                nc.tensor.matmul(
                    pwarm, junk[:, 0:128], junk[:, 128:640], start=True, stop=True
                )

            # ---------- projection emitters (per q/k/v tile m-block) -------
            def load_x_tiles(t):
                """DMA the x operand tiles for projection tile t (v first —
                its units run first). q/k arrive as fp8 hi/lo slot pairs."""
                tsl = ts(t, QT)
                tiles = {}
                xv = xs.tile([128, 8, QT], F16, name="xv", tag="xv", bufs=2)
                for kc in range(8):
                    nc.sync.dma_start(out=xv[:, kc, :], in_=vT_d[ts(kc, 128), tsl])
                tiles["v"] = xv
                for key, src_d in (("k", xk8_d), ("q", xq8_d)):
                    x = xs.tile(
                        [128, 8, 2, QT], F8, name=f"x{key}", tag=f"x{key}", bufs=2
                    )
                    for kc in range(8):
                        nc.sync.dma_start(out=x[:, kc, :, :], in_=src_d[kc, :, :, tsl])
                    tiles[key] = x
                return tiles

            def proj_qk_unit(which, x, t, m):
                """One m-block of the q or k projection: fp8 hi/lo DoubleRow
                matmuls, cast to fp8, regroup-DMA into DoubleRow layout."""
                tsl = ts(t, QT)
                w_sb = wq8_sb if which == "q" else wk8_sb
                nat = qT8 if which == "q" else kT8
                dr = qdr if which == "q" else kdr
                pj = ps.tile([128, QT], F32, name="pj", tag="pj", bufs=2)
                for kc in range(8):
                    nc.tensor.matmul(
                        pj,
                        w_sb[:, kc, :, ts(m, 128)],
                        x[:, kc, :, :],
                        start=(kc == 0),
                        stop=(kc == 7),
                        perf_mode=DR,
                    )
                nc.vector.tensor_copy(nat[:, m, tsl], pj)
                nc.sync.dma_start(out=dr[0:64, m, :, tsl], in_=nat[:, m, tsl])

            def proj_v_unit(x, t, m):
                """One m-block of the v projection (natural [seq, d] layout
                plus the ones column feeding softmax denominators)."""
                pj = ps.tile([128, QT], F32, name="pj", tag="pj", bufs=2)
                for kc in range(8):
                    nc.tensor.matmul(
                        pj,
                        x[:, kc, ts(m, 128)],
                        wv_sb[:, kc, :],
                        start=(kc == 0),
                        stop=(kc == 7),
                    )
                sb = t * 4 + m
                nc.vector.tensor_copy(
                    v_all[:, sb, :, 0:64],
                    pj[:, :].rearrange("p (h d) -> p h d", h=8),
                )
                nc.vector.tensor_copy(v_all[:, sb, :, 64], onescol_sb)

            def proj_tile_units(t, x=None):
                """Filler units projecting tile t (v first: attention tile t
                consumes v_all[4t..4t+3] earliest via PV j=4t)."""
                if x is None:
                    x = load_x_tiles(t)
                units = []
                for m in range(4):
                    units.append(lambda m=m, x=x["v"]: proj_v_unit(x, t, m))
                for m in range(4):
                    units.append(lambda m=m, x=x["k"]: proj_qk_unit("k", x, t, m))
                for m in range(4):
                    units.append(lambda m=m, x=x["q"]: proj_qk_unit("q", x, t, m))
                return units

            # ---------- normalize + O-projection emitters ------------------
            def norm_unit(t):
                last = t == NT - 1
                if last:
                    # rows 0:6 (head pairs g=0..2) were reciprocal'd early,
                    # inside the ladder; only g=3's rows remain on the tail
                    nc.vector.reciprocal(recip3b, rs3b)
                else:
                    nc.vector.reciprocal(recip_sb[t], rs_sb[t])
                for g in range(4):
                    bc = ps.tile([128, QT], F32, name="bc", tag="pj", bufs=2)
                    if last and g == 3:
                        nc.tensor.matmul(
                            bc, sel2_sb[:, :], recip3b[:, :],
                            start=True, stop=True,
                        )
                    elif last:
                        nc.tensor.matmul(
                            bc, sel6_sb[:, g, :], recip_sb[t][0:6, :],
                            start=True, stop=True,
                        )
                    else:
                        nc.tensor.matmul(
                            bc, sel_sb[:, g, :], recip_sb[t][:, :],
                            start=True, stop=True,
                        )
                    nc.vector.tensor_mul(ot_sb[t][:, g, :], ot_sb[t][:, g, :], bc)

            def oproj_unit(t, mm, n2):
                po = ps.tile([128, 512], F32, name="po", tag="pj", bufs=2)
                for g in range(4):
                    nc.tensor.matmul(
                        po,
                        ot_sb[t][:, g, ts(mm, 128)],
                        wo_sb[:, g, ts(n2, 512)],
                        start=(g == 0),
                        stop=(g == 3),
                    )
                ob = outp.tile([128, 512], F32, name="ob", tag="ob", bufs=3)
                nc.vector.tensor_copy(ob, po)
                nc.sync.dma_start(
                    out=out_d[ts(4 * t + mm, 128), ts(n2, 512)], in_=ob
                )

            def oproj_tile_units(t):
                units = [lambda: norm_unit(t)]
                for mm in range(4):
                    for n2 in range(2):
                        units.append(
                            lambda mm=mm, n2=n2: oproj_unit(t, mm, n2)
                        )
                return units

            # ================= main schedule =================
            # proj(t0) up front; proj(t+1) and oproj(t-1) woven into the
            # attention ladder as PE filler between blocks.
            filler = deque()
            # tile-0 x DMAs go out before the q/k/o weight DMAs so the
            # v-projection (first PE work after warmup) is fed earliest
            x0 = load_x_tiles(0)
            for kc in range(8):
                nc.sync.dma_start(out=wq8_sb[:, kc, :, :], in_=wq8_d[kc, :, :, :])
                nc.sync.dma_start(out=wk8_sb[:, kc, :, :], in_=wk8_d[kc, :, :, :])
            for g in range(4):
                nc.sync.dma_start(out=wo_sb[:, g, :], in_=woT_d[ts(g, 128), :])
            for u in proj_tile_units(0, x0):
                u()

            for t in range(NT):
                nkb = 4 * t + 4  # causal: key blocks 0 .. 4t+3
                if t + 1 < NT:
                    filler.extend(proj_tile_units(t + 1))
                if t > 0:
                    filler.extend(oproj_tile_units(t - 1))
                blocks_left = 4 * nkb
                stride = max(1, (4 * nkb) // max(1, len(filler)))
                since = 0
                for g in range(4):
                    ota = ps.tile([65, QT], F32, name="ota", tag="ota", bufs=1)
                    otb = ps.tile([65, QT], F32, name="otb", tag="otb", bufs=1)
                    for j in range(nkb):
                        r = j - 4 * t
                        z = 128 * r if r > 0 else 0
                        sp = ps.tile(
                            [128, 2, QT], F32, name="sp", tag="sp", bufs=2
                        )
                        for b2 in range(2):
                            pb = 32 * b2
                            nc.tensor.matmul(
                                sp[:, b2, z:QT],
                                kdr[pb : pb + 32, g, :, ts(j, 128)],
                                qdr[pb : pb + 32, g, :, t * QT + z : (t + 1) * QT],
                                start=True,
                                stop=True,
                                perf_mode=DR,
                            )
                        pt2 = pt_pool.tile(
                            [128, 2, QT], F16, name="pt2", tag="pt2", bufs=6
                        )
                        # scores carry the host-side 64x w_q/w_k pre-scales
                        nc.scalar.activation(
                            pt2[:, :, z:QT], sp[:, :, z:QT], AF.Exp,
                            scale=0.125 / 4096.0,
                        )
                        if r >= 0:
                            nc.gpsimd.tensor_mul(
                                pt2[:, :, z : z + 128],
                                pt2[:, :, z : z + 128],
                                maskw_sb,
                            )
                        nc.tensor.matmul(
                            ota[:, z:QT],
                            v_all[:, j, 2 * g, :],
                            pt2[:, 0, z:QT],
                            start=(j == 0),
                            stop=(j == nkb - 1),
                        )
                        nc.tensor.matmul(
                            otb[:, z:QT],
                            v_all[:, j, 2 * g + 1, :],
                            pt2[:, 1, z:QT],
                            start=(j == 0),
                            stop=(j == nkb - 1),
                        )
                        # weave pending proj/oproj PE work between blocks,
                        # spread evenly and guaranteed drained by tile end
                        since += 1
                        if filler and since >= stride:
                            filler.popleft()()
                            since = 0
                        while filler and len(filler) >= blocks_left:
                            filler.popleft()()
                        blocks_left -= 1
                    # stage O^T and rowsums to SBUF
                    nc.vector.tensor_copy(ot_sb[t][0:64, g, :], ota[0:64, :])
                    nc.vector.tensor_copy(ot_sb[t][64:128, g, :], otb[0:64, :])
                    tmp_rs = pt_pool.tile(
                        [1, 2, QT], F32, name="tmp_rs", tag="tmp_rs", bufs=2
                    )
                    nc.vector.tensor_copy(tmp_rs[0:1, 0, :], ota[64:65, :])
                    nc.vector.tensor_copy(tmp_rs[0:1, 1, :], otb[64:65, :])
                    if t == NT - 1 and g == 3:
                        nc.sync.dma_start(out=rs3b[:, :], in_=tmp_rs[0:1, :, :])
                    else:
                        nc.sync.dma_start(
                            out=rs_sb[t][2 * g : 2 * g + 2, :],
                            in_=tmp_rs[0:1, :, :],
                        )
                    if t == NT - 1 and g == 2:
                        # take g=0..2's reciprocals off the final-tile tail
                        nc.vector.reciprocal(
                            recip_sb[t][0:6, :], rs_sb[t][0:6, :]
                        )
            for u in filler:
                u()
            for u in oproj_tile_units(NT - 1):
                u()

    _split_sync_waits(nc)
    return nc


def _prep_inputs(Q, K, V, w_q, w_k, w_v, w_o):
    """Build the 8 per-core input maps (host-side shard + transpose + cast)."""
    Q = np.asarray(Q, dtype=np.float32)
    K = np.asarray(K, dtype=np.float32)
    V = np.asarray(V, dtype=np.float32)
    w_q = np.asarray(w_q, dtype=np.float32)
    w_k = np.asarray(w_k, dtype=np.float32)
    w_v = np.asarray(w_v, dtype=np.float32)
    w_o = np.asarray(w_o, dtype=np.float32)

    k_idx = np.arange(128)[:, None]
    w_idx = np.arange(128)[None, :]
    maskw = np.zeros((128, 2, 128), dtype=np.float16)
    maskw[:, 0, :] = (k_idx <= w_idx).astype(np.float16)
    maskw[:, 1, :] = maskw[:, 0, :]
    onescol = np.ones((128, 8), dtype=np.float16)
    sel = np.zeros((8, 4, 128), dtype=np.float16)
    for g in range(4):
        sel[2 * g, g, 0:64] = 1.0
        sel[2 * g + 1, g, 64:128] = 1.0
    sel6 = np.zeros((6, 3, 128), dtype=np.float16)
    for g in range(3):
        sel6[2 * g, g, 0:64] = 1.0
        sel6[2 * g + 1, g, 64:128] = 1.0
    sel2 = np.zeros((2, 128), dtype=np.float16)
    sel2[0, 0:64] = 1.0
    sel2[1, 64:128] = 1.0

    import concourse.mybir as mybir

    F8NP = mybir.dt.np(mybir.dt.float8e4)

    def f16T(a):
        return np.ascontiguousarray(a.T).astype(np.float16)

    def x8_hilo(x):
        # [1024, N] -> [8, 128, 2, N] fp8 with slots (x8, 16*(x - x8))
        xT = np.ascontiguousarray(x.T, dtype=np.float32)
        hi = xT.astype(F8NP)
        lo = (16.0 * (xT - hi.astype(np.float32))).astype(F8NP)
        out = np.empty((8, 128, 2, xT.shape[1]), dtype=F8NP)
        out[:, :, 0, :] = hi.reshape(8, 128, -1)
        out[:, :, 1, :] = lo.reshape(8, 128, -1)
        return out

    def w8_hilo(w, hg):
        # [8, 128, 2, 512] fp8 with slots (w8, w8/16); w pre-scaled by 64
        wT = np.ascontiguousarray(
            w[hg * 512 : hg * 512 + 512, :].T * 64.0, dtype=np.float32
        )
        hi = wT.astype(F8NP)
        lo = (hi.astype(np.float32) / 16.0).astype(F8NP)
        out = np.empty((8, 128, 2, 512), dtype=F8NP)
        out[:, :, 0, :] = hi.reshape(8, 128, -1)
        out[:, :, 1, :] = lo.reshape(8, 128, -1)
        return out

    xq8 = [x8_hilo(Q[b]) for b in range(B)]
    xk8 = [x8_hilo(K[b]) for b in range(B)]
    vT = [f16T(V[b]) for b in range(B)]
    wq8 = [w8_hilo(w_q, hg) for hg in range(2)]
    wk8 = [w8_hilo(w_k, hg) for hg in range(2)]
    wvT = [f16T(w_v[hg * 512 : hg * 512 + 512, :]) for hg in range(2)]
    woT = [f16T(w_o[:, hg * 512 : hg * 512 + 512]) for hg in range(2)]

    in_maps = []
    for c in range(N_CORES):
        b, hg = c // 2, c % 2
        in_maps.append(
            {
                "xq8": xq8[b],
                "xk8": xk8[b],
                "vT": vT[b],
                "wq8": wq8[hg],
                "wk8": wk8[hg],
                "wvT": wvT[hg],
                "woT": woT[hg],
                "maskw": maskw,
                "onescol": onescol,
                "sel": sel,
                "sel6": sel6,
                "sel2": sel2,
            }
        )
    return in_maps


def kernel(Q, K, V, w_q, w_k, w_v, w_o, _trace=False):
    from concourse.bass_utils import run_bass_kernel_spmd

    if "nc" not in _CACHE:
        _CACHE["nc"] = build_nc()
    nc = _CACHE["nc"]

    in_maps = _prep_inputs(Q, K, V, w_q, w_k, w_v, w_o)
    res = run_bass_kernel_spmd(
        nc, in_maps, core_ids=list(range(N_CORES)), trace=_trace
    )
    outs = [r["out"] for r in res.results]
    full = np.empty((B, N, D_MODEL), dtype=np.float32)
    for b in range(B):
        full[b] = outs[2 * b] + outs[2 * b + 1]
    if _trace:
        _CACHE["last_result"] = res
    return full


# revision 4
# speedup vs baseline: 1.1056x; 1.0088x over previous
"""Multi-head causal attention (b=4, n=2048, d_model=1024, 16 heads) on 8
Trainium2 NeuronCores.

Sharding: core c = (batch b = c//2, head-group hg = c%2); each core computes
one batch with 8 heads (tensor-parallel split of w_q/w_k/w_v by rows and w_o
by columns) and returns a partial [2048, 1024] output; host sums the two
head-group partials per batch.

v2 vs baseline:
- Scores run in fp8e4 DoubleRow mode (0.5 PE cycles/row): q/k projections
  stay fp16 for accuracy, but their outputs are cast straight to fp8 and
  regrouped (flat sbuf->sbuf DMA, [128,512] -> [64,2,512]) into the
  DoubleRow pairing d = 2p + i.
- Scores/exp/PV are trimmed to the causal window on diagonal blocks
  (baseline only trimmed PV).
- The causal mask multiply runs on the idle GpSimd engine against a single
  [128,2,128] triangular window instead of DVE x [128,1024].
- Projection and O-projection PE work is woven into the attention ladder
  as filler units so the PE never idles waiting on exp; ACT (exp) and PE
  stay concurrently busy instead of phase-serialized.
"""

from collections import deque

import numpy as np

B = 4
N = 2048
D_MODEL = 1024
DK = 64
NT = 4          # q tiles of 512
QT = 512        # q tile size
N_CORES = 8

_CACHE = {}


def _split_sync_waits(nc, max_waits=1):
    """walrus on this image allows only 1 sync-wait command per instruction;
    hoist excess waits onto same-engine NoOps inserted just before."""
    import concourse.mybir as mybir

    n_split = 0
    for fn in nc.m.functions:
        for blk in fn.blocks:
            insts = list(blk.instructions)
            out = []
            for inst in insts:
                si = inst.sync_info
                if si is not None and len(si.on_wait) > max_waits:
                    waits = list(si.on_wait)
                    head, rest = waits[:-max_waits], waits[-max_waits:]
                    while head:
                        chunk, head = head[:max_waits], head[max_waits:]
                        nop = mybir.InstNoOp(
                            name=f"{inst.name}-ws{n_split}-{len(out)}",
                            engine=inst.engine,
                            opcode="NoOp",
                            sync_info=mybir.SyncInfo(on_wait=chunk, on_update=[]),
                            bass_nofuse=True,
                        )
                        out.append(nop)
                    si.on_wait = rest
                    n_split += 1
                out.append(inst)
            if len(out) != len(insts):
                blk.instructions = out
    return n_split


def build_nc():
    import concourse.bass as bass
    import concourse.mybir as mybir
    import concourse.tile as tile
    from concourse.bass import ts

    F32 = mybir.dt.float32
    F16 = mybir.dt.float16
    F8 = mybir.dt.float8e4
    AF = mybir.ActivationFunctionType
    DR = mybir.MatmulPerfMode.DoubleRow

    nc = bass.Bass("TRN2", target_bir_lowering=False, debug=False)

    qT_d = nc.dram_tensor("qT", [D_MODEL, N], F16, kind="ExternalInput")
    kT_d = nc.dram_tensor("kT", [D_MODEL, N], F16, kind="ExternalInput")
    vT_d = nc.dram_tensor("vT", [D_MODEL, N], F16, kind="ExternalInput")
    wqT_d = nc.dram_tensor("wqT", [D_MODEL, 512], F16, kind="ExternalInput")
    wkT_d = nc.dram_tensor("wkT", [D_MODEL, 512], F16, kind="ExternalInput")
    wvT_d = nc.dram_tensor("wvT", [D_MODEL, 512], F16, kind="ExternalInput")
    woT_d = nc.dram_tensor("woT", [512, D_MODEL], F16, kind="ExternalInput")
    maskw_d = nc.dram_tensor("maskw", [128, 2, 128], F16, kind="ExternalInput")
    onescol_d = nc.dram_tensor("onescol", [128, 8], F16, kind="ExternalInput")
    sel_d = nc.dram_tensor("sel", [8, 4, 128], F16, kind="ExternalInput")
    sel6_d = nc.dram_tensor("sel6", [6, 3, 128], F16, kind="ExternalInput")
    sel2_d = nc.dram_tensor("sel2", [2, 128], F16, kind="ExternalInput")
    out_d = nc.dram_tensor("out", [N, D_MODEL], F32, kind="ExternalOutput")

    with (
        tile.TileContext(nc) as tc,
        nc.allow_low_precision(reason="fp8/fp16 matmuls are intentional"),
    ):
        with (
            tc.tile_pool(name="persist", bufs=1) as persist,
            tc.tile_pool(name="pt_pool", bufs=1) as pt_pool,
            tc.tile_pool(name="xs", bufs=2) as xs,
            tc.tile_pool(name="outp", bufs=1) as outp,
            tc.tile_pool(name="ps", bufs=1, space="PSUM") as ps,
        ):
            # ---- persistent SBUF tensors ----
            # q/k in fp16, heads packed 2-per-partition-group: head pair g
            # lives at [0:64] (even head) / [64:128] (odd head), plane g
            qT_all = persist.tile([128, 4, N], F16)
            kT_all = persist.tile([128, 4, N], F16)
            v_all = persist.tile([128, 16, 8, 65], F16)  # [key, sb, head, d+1]
            maskw_sb = persist.tile([128, 2, 128], F16)
            onescol_sb = persist.tile([128, 8], F16)
            sel_sb = persist.tile([8, 4, 128], F16)
            sel6_sb = persist.tile([6, 3, 128], F16)
            sel2_sb = persist.tile([2, 128], F16)
            rs3b = persist.tile([2, QT], F32)
            recip3b = persist.tile([2, QT], F16)
            wq_sb = persist.tile([128, 8, 512], F16)
            wk_sb = persist.tile([128, 8, 512], F16)
            wv_sb = persist.tile([128, 8, 512], F16)
            wo_sb = persist.tile([128, 4, D_MODEL], F16)
            ot_sb = [
                persist.tile([128, 4, QT], F16, name=f"ot_sb{t}", tag=f"ot{t}")
                for t in range(NT)
            ]
            rs_sb = [
                persist.tile([8, QT], F32, name=f"rs_sb{t}", tag=f"rs{t}")
                for t in range(NT)
            ]
            recip_sb = [
                persist.tile([8, QT], F16, name=f"recip{t}", tag=f"rc{t}")
                for t in range(NT)
            ]

            # DMA order matters at startup: the v-projection path (wv + vT
            # tile 0) is needed first; wo only at the first O-projection.
            for kc in range(8):
                nc.sync.dma_start(out=wv_sb[:, kc, :], in_=wvT_d[ts(kc, 128), :])
            nc.sync.dma_start(out=maskw_sb, in_=maskw_d[:, :, :])
            nc.sync.dma_start(out=onescol_sb, in_=onescol_d[:, :])
            nc.sync.dma_start(out=sel_sb, in_=sel_d[:, :, :])
            nc.sync.dma_start(out=sel6_sb, in_=sel6_d[:, :, :])
            nc.sync.dma_start(out=sel2_sb, in_=sel2_d[:, :])

            # ---- PE p-state warmup while initial DMAs land ----
            junk = persist.tile([128, 640], F16)
            nc.vector.memset(junk, 0.0)
            pwarm = ps.tile([128, QT], F32, name="pwarm", tag="pj", bufs=2)
            for _ in range(14):
                nc.tensor.matmul(
                    pwarm, junk[:, 0:128], junk[:, 128:640], start=True, stop=True
                )

            # ---------- projection emitters (per q/k/v tile m-block) -------
            def load_x_tiles(t):
                """DMA the x operand tiles for projection tile t (v first —
                its units run first)."""
                tsl = ts(t, QT)
                tiles = {}
                for key, src_d in (("v", vT_d), ("k", kT_d), ("q", qT_d)):
                    x = xs.tile(
                        [128, 8, QT], F16, name=f"x{key}", tag=f"x{key}", bufs=2
                    )
                    for kc in range(8):
                        nc.sync.dma_start(out=x[:, kc, :], in_=src_d[ts(kc, 128), tsl])
                    tiles[key] = x
                return tiles

            def proj_qk_unit(which, x, t, m):
                """One m-block of the q or k projection (fp16)."""
                tsl = ts(t, QT)
                w_sb = wq_sb if which == "q" else wk_sb
                dst = qT_all if which == "q" else kT_all
                pj = ps.tile([128, QT], F32, name="pj", tag="pj", bufs=2)
                for kc in range(8):
                    nc.tensor.matmul(
                        pj,
                        w_sb[:, kc, ts(m, 128)],
                        x[:, kc, :],
                        start=(kc == 0),
                        stop=(kc == 7),
                    )
                nc.vector.tensor_copy(dst[:, m, tsl], pj)

            def proj_v_unit(x, t, m):
                """One m-block of the v projection (natural [seq, d] layout
                plus the ones column feeding softmax denominators)."""
                pj = ps.tile([128, QT], F32, name="pj", tag="pj", bufs=2)
                for kc in range(8):
                    nc.tensor.matmul(
                        pj,
                        x[:, kc, ts(m, 128)],
                        wv_sb[:, kc, :],
                        start=(kc == 0),
                        stop=(kc == 7),
                    )
                sb = t * 4 + m
                nc.vector.tensor_copy(
                    v_all[:, sb, :, 0:64],
                    pj[:, :].rearrange("p (h d) -> p h d", h=8),
                )
                nc.vector.tensor_copy(v_all[:, sb, :, 64], onescol_sb)

            def proj_tile_units(t, x=None):
                """Filler units projecting tile t (v first: attention tile t
                consumes v_all[4t..4t+3] earliest via PV j=4t)."""
                if x is None:
                    x = load_x_tiles(t)
                units = []
                for m in range(4):
                    units.append(lambda m=m, x=x["v"]: proj_v_unit(x, t, m))
                for m in range(4):
                    units.append(lambda m=m, x=x["k"]: proj_qk_unit("k", x, t, m))
                for m in range(4):
                    units.append(lambda m=m, x=x["q"]: proj_qk_unit("q", x, t, m))
                return units

            # ---------- normalize + O-projection emitters ------------------
            def norm_unit(t):
                last = t == NT - 1
                if last:
                    # rows 0:6 (head pairs g=0..2) were reciprocal'd early,
                    # inside the ladder; only g=3's rows remain on the tail
                    nc.vector.reciprocal(recip3b, rs3b)
                else:
                    nc.vector.reciprocal(recip_sb[t], rs_sb[t])
                for g in range(4):
                    bc = ps.tile([128, QT], F32, name="bc", tag="pj", bufs=2)
                    if last and g == 3:
                        nc.tensor.matmul(
                            bc, sel2_sb[:, :], recip3b[:, :],
                            start=True, stop=True,
                        )
                    elif last:
                        nc.tensor.matmul(
                            bc, sel6_sb[:, g, :], recip_sb[t][0:6, :],
                            start=True, stop=True,
                        )
                    else:
                        nc.tensor.matmul(
                            bc, sel_sb[:, g, :], recip_sb[t][:, :],
                            start=True, stop=True,
                        )
                    nc.vector.tensor_mul(ot_sb[t][:, g, :], ot_sb[t][:, g, :], bc)

            def oproj_unit(t, mm, n2):
                po = ps.tile([128, 512], F32, name="po", tag="pj", bufs=2)
                for g in range(4):
                    nc.tensor.matmul(
                        po,
                        ot_sb[t][:, g, ts(mm, 128)],
                        wo_sb[:, g, ts(n2, 512)],
                        start=(g == 0),
                        stop=(g == 3),
                    )
                ob = outp.tile([128, 512], F32, name="ob", tag="ob", bufs=3)
                nc.vector.tensor_copy(ob, po)
                nc.sync.dma_start(
                    out=out_d[ts(4 * t + mm, 128), ts(n2, 512)], in_=ob
                )

            def oproj_tile_units(t):
                units = [lambda: norm_unit(t)]
                for mm in range(4):
                    for n2 in range(2):
                        units.append(
                            lambda mm=mm, n2=n2: oproj_unit(t, mm, n2)
                        )
                return units

            # ================= main schedule =================
            # proj(t0) up front; proj(t+1) and oproj(t-1) woven into the
            # attention ladder as PE filler between blocks.
            filler = deque()
            # tile-0 x DMAs go out before the q/k/o weight DMAs so the
            # v-projection (first PE work after warmup) is fed earliest
            x0 = load_x_tiles(0)
            for kc in range(8):
                nc.sync.dma_start(out=wq_sb[:, kc, :], in_=wqT_d[ts(kc, 128), :])
                nc.sync.dma_start(out=wk_sb[:, kc, :], in_=wkT_d[ts(kc, 128), :])
            for g in range(4):
                nc.sync.dma_start(out=wo_sb[:, g, :], in_=woT_d[ts(g, 128), :])
            for u in proj_tile_units(0, x0):
                u()

            for t in range(NT):
                nkb = 4 * t + 4  # causal: key blocks 0 .. 4t+3
                if t + 1 < NT:
                    filler.extend(proj_tile_units(t + 1))
                if t > 0:
                    filler.extend(oproj_tile_units(t - 1))
                blocks_left = 4 * nkb
                stride = max(1, (4 * nkb) // max(1, len(filler)))
                since = 0
                for g in range(4):
                    ota = ps.tile([65, QT], F32, name="ota", tag="ota", bufs=1)
                    otb = ps.tile([65, QT], F32, name="otb", tag="otb", bufs=1)

                    sp_live = {}

                    def emit_scores(j, t=t, g=g, sp_live=sp_live):
                        r = j - 4 * t
                        z = 128 * r if r > 0 else 0
                        sp = ps.tile(
                            [128, 2, QT], F32, name="sp", tag="sp", bufs=2
                        )
                        for b2 in range(2):
                            pb = 64 * b2
                            nc.tensor.matmul(
                                sp[:, b2, z:QT],
                                kT_all[pb : pb + 64, g, ts(j, 128)],
                                qT_all[pb : pb + 64, g, t * QT + z : (t + 1) * QT],
                                start=True,
                                stop=True,
                                tile_position=(pb, 0),
                            )
                        sp_live[j] = (sp, z)

                    # scores run one block ahead of exp/PV so the PE never
                    # sits on the exp dependency
                    emit_scores(0)
                    for j in range(nkb):
                        if j + 1 < nkb:
                            emit_scores(j + 1)
                        sp, z = sp_live.pop(j)
                        r = j - 4 * t
                        pt2 = pt_pool.tile(
                            [128, 2, QT], F16, name="pt2", tag="pt2", bufs=6
                        )
                        nc.scalar.activation(
                            pt2[:, :, z:QT], sp[:, :, z:QT], AF.Exp, scale=0.125
                        )
                        if r >= 0:
                            nc.gpsimd.tensor_mul(
                                pt2[:, :, z : z + 128],
                                pt2[:, :, z : z + 128],
                                maskw_sb,
                            )
                        nc.tensor.matmul(
                            ota[:, z:QT],
                            v_all[:, j, 2 * g, :],
                            pt2[:, 0, z:QT],
                            start=(j == 0),
                            stop=(j == nkb - 1),
                        )
                        nc.tensor.matmul(
                            otb[:, z:QT],
                            v_all[:, j, 2 * g + 1, :],
                            pt2[:, 1, z:QT],
                            start=(j == 0),
                            stop=(j == nkb - 1),
                        )
                        # weave pending proj/oproj PE work between blocks,
                        # spread evenly and guaranteed drained by tile end
                        since += 1
                        if filler and since >= stride:
                            filler.popleft()()
                            since = 0
                        while filler and len(filler) >= blocks_left:
                            filler.popleft()()
                        blocks_left -= 1
                    # stage O^T and rowsums to SBUF
                    nc.vector.tensor_copy(ot_sb[t][0:64, g, :], ota[0:64, :])
                    nc.vector.tensor_copy(ot_sb[t][64:128, g, :], otb[0:64, :])
                    tmp_rs = pt_pool.tile(
                        [1, 2, QT], F32, name="tmp_rs", tag="tmp_rs", bufs=2
                    )
                    nc.vector.tensor_copy(tmp_rs[0:1, 0, :], ota[64:65, :])
                    nc.vector.tensor_copy(tmp_rs[0:1, 1, :], otb[64:65, :])
                    if t == NT - 1 and g == 3:
                        nc.sync.dma_start(out=rs3b[:, :], in_=tmp_rs[0:1, :, :])
                    else:
                        nc.sync.dma_start(
                            out=rs_sb[t][2 * g : 2 * g + 2, :],
                            in_=tmp_rs[0:1, :, :],
                        )
                    if t == NT - 1 and g == 2:
                        # take g=0..2's reciprocals off the final-tile tail
                        nc.vector.reciprocal(
                            recip_sb[t][0:6, :], rs_sb[t][0:6, :]
                        )
            for u in filler:
                u()
            for u in oproj_tile_units(NT - 1):
                u()

    _split_sync_waits(nc)
    return nc


def _prep_inputs(Q, K, V, w_q, w_k, w_v, w_o):
    """Build the 8 per-core input maps (host-side shard + transpose + cast)."""
    Q = np.asarray(Q, dtype=np.float32)
    K = np.asarray(K, dtype=np.float32)
    V = np.asarray(V, dtype=np.float32)
    w_q = np.asarray(w_q, dtype=np.float32)
    w_k = np.asarray(w_k, dtype=np.float32)
    w_v = np.asarray(w_v, dtype=np.float32)
    w_o = np.asarray(w_o, dtype=np.float32)

    k_idx = np.arange(128)[:, None]
    w_idx = np.arange(128)[None, :]
    maskw = np.zeros((128, 2, 128), dtype=np.float16)
    maskw[:, 0, :] = (k_idx <= w_idx).astype(np.float16)
    maskw[:, 1, :] = maskw[:, 0, :]
    onescol = np.ones((128, 8), dtype=np.float16)
    sel = np.zeros((8, 4, 128), dtype=np.float16)
    for g in range(4):
        sel[2 * g, g, 0:64] = 1.0
        sel[2 * g + 1, g, 64:128] = 1.0
    sel6 = np.zeros((6, 3, 128), dtype=np.float16)
    for g in range(3):
        sel6[2 * g, g, 0:64] = 1.0
        sel6[2 * g + 1, g, 64:128] = 1.0
    sel2 = np.zeros((2, 128), dtype=np.float16)
    sel2[0, 0:64] = 1.0
    sel2[1, 64:128] = 1.0

    def f16T(a):
        return np.ascontiguousarray(a.T).astype(np.float16)

    qT = [f16T(Q[b]) for b in range(B)]
    kT = [f16T(K[b]) for b in range(B)]
    vT = [f16T(V[b]) for b in range(B)]
    wqT = [f16T(w_q[hg * 512 : hg * 512 + 512, :]) for hg in range(2)]
    wkT = [f16T(w_k[hg * 512 : hg * 512 + 512, :]) for hg in range(2)]
    wvT = [f16T(w_v[hg * 512 : hg * 512 + 512, :]) for hg in range(2)]
    woT = [f16T(w_o[:, hg * 512 : hg * 512 + 512]) for hg in range(2)]

    in_maps = []
    for c in range(N_CORES):
        b, hg = c // 2, c % 2
        in_maps.append(
            {
                "qT": qT[b],
                "kT": kT[b],
                "vT": vT[b],
                "wqT": wqT[hg],
                "wkT": wkT[hg],
                "wvT": wvT[hg],
                "woT": woT[hg],
                "maskw": maskw,
                "onescol": onescol,
                "sel": sel,
                "sel6": sel6,
                "sel2": sel2,
            }
        )
    return in_maps


def kernel(Q, K, V, w_q, w_k, w_v, w_o, _trace=False):
    from concourse.bass_utils import run_bass_kernel_spmd

    if "nc" not in _CACHE:
        _CACHE["nc"] = build_nc()
    nc = _CACHE["nc"]

    in_maps = _prep_inputs(Q, K, V, w_q, w_k, w_v, w_o)
    res = run_bass_kernel_spmd(
        nc, in_maps, core_ids=list(range(N_CORES)), trace=_trace
    )
    outs = [r["out"] for r in res.results]
    full = np.empty((B, N, D_MODEL), dtype=np.float32)
    for b in range(B):
        full[b] = outs[2 * b] + outs[2 * b + 1]
    if _trace:
        _CACHE["last_result"] = res
    return full
